# revision 4
# baseline (speedup 1.0000x reference)
"""Embedding lookup (disguised as one-hot @ W.T + b) on 8 TRN2 NeuronCores.

Reference computes out[b,s,:] = W[:, src[b,s]] + b with
  src: [16, 256] int, W: [128, 32000] f32, b: [128] f32  ->  out [16, 256, 128] f32.

Strategy (data-parallel on batch, per the sharding hint):
  - Host: relayout W to a row-major table W_T = W.T  [V=32000, H=128] so one
    embedding row is 512 contiguous bytes; replicate W_T + bias to all cores.
  - Each of the 8 cores handles 512 tokens (2 batches). Token order is
    permuted on host so gather slot (p, j) holds token 4*p + j, making the
    final store per-partition contiguous (2 KB / partition).
  - Device: one SWDGE dma_gather pulls all 512 rows HBM->SBUF (512 B
    descriptors), one DVE add applies the bias, one HWDGE DMA stores the
    [128, 4*128] tile to the output.
"""

import sys

import numpy as np

if "/opt/trn_rl_repo" not in sys.path:
    sys.path.insert(0, "/opt/trn_rl_repo")

B, S, V, H = 16, 256, 32000, 128
N_CORES = 8
TOK = B * S // N_CORES  # 512 tokens per core
J = TOK // 128  # 4 tokens per partition
IDX_COLS = TOK // 16  # 32 int16 per partition row in the wrapped index tile

_NC_CACHE = {}


def _build_nc():
    import concourse.bacc as bacc
    import concourse.mybir as mybir

    nc = bacc.Bacc("TRN2", target_bir_lowering=False)

    wt = nc.dram_tensor("wt", [V, H], mybir.dt.float32, kind="ExternalInput")
    idx = nc.dram_tensor("idx", [128, IDX_COLS], mybir.dt.int16, kind="ExternalInput")
    bias = nc.dram_tensor("bias", [128, J * H], mybir.dt.float32, kind="ExternalInput")
    out = nc.dram_tensor("out", [TOK, H], mybir.dt.float32, kind="ExternalOutput")
    # token t = 4p + j lives at dst[p, j, :]; store is contiguous per partition.
    out_view = out[:].rearrange("(p j) h -> p (j h)", p=128)

    with (
        nc.sbuf_tensor("idx_sb", [128, IDX_COLS], mybir.dt.int16) as idx_sb,
        nc.sbuf_tensor("warm_idx", [128, 8], mybir.dt.int16) as warm_idx,
        nc.sbuf_tensor("warm_dst", [128, 1, H], mybir.dt.float32) as warm_dst,
        nc.sbuf_tensor("dst_sb", [128, J, H], mybir.dt.float32) as dst_sb,
        nc.sbuf_tensor("bias_sb", [128, J * H], mybir.dt.float32) as bias_sb,
        nc.semaphore("s_idx") as s_idx,
        nc.semaphore("s_bias") as s_bias,
        nc.semaphore("s_warm") as s_warm,
        nc.semaphore("s_g") as s_g,
        nc.semaphore("s_v") as s_v,
        nc.semaphore("s_out") as s_out,
        nc.Block() as block,
    ):
        dst_flat = dst_sb[:].rearrange("p j h -> p (j h)")

        @block.sync
        def _(sync):
            sync.dma_start(idx_sb[:], idx[:]).then_inc(s_idx, 16)
            sync.dma_start(bias_sb[:], bias[:]).then_inc(s_bias, 16)
            sync.wait_ge(s_v, 1)
            sync.dma_start(out_view, dst_flat).then_inc(s_out, 16)
            sync.wait_ge(s_out, 16)

        @block.gpsimd
        def _(gpsimd):
            # Warmup: force the mlp Q7 library load + its ~6us IRAM fetch to
            # happen while the input DMAs are still in flight (all-zero
            # indices gather row 0 into a scratch tile).
            gpsimd.memset(warm_idx[:], 0)
            gpsimd.dma_gather(
                warm_dst[:], wt[:], warm_idx[:], 128, 128, H, single_packet=False
            ).then_inc(s_warm, 16)
            gpsimd.wait_ge(s_idx, 16)
            gpsimd.dma_gather(
                dst_sb[:], wt[:], idx_sb[:], TOK, TOK, H, single_packet=False
            ).then_inc(s_g, 16)

        @block.vector
        def _(vector):
            vector.wait_ge(s_bias, 16)
            vector.wait_ge(s_g, 16)
            vector.tensor_add(dst_flat, dst_flat, bias_sb[:]).then_inc(s_v, 1)

    nc.compile()
    return nc


def _run(src, W, b, **spmd_kwargs):
    from concourse.bass_utils import run_bass_kernel_spmd

    src = np.asarray(src)
    W = np.asarray(W, dtype=np.float32)
    b = np.asarray(b, dtype=np.float32)
    assert src.shape == (B, S) and W.shape == (H, V) and b.shape == (H,)

    if "nc" not in _NC_CACHE:
        _NC_CACHE["nc"] = _build_nc()
    nc = _NC_CACHE["nc"]

    # Host-side sharding / layout prep.
    w_t = np.ascontiguousarray(W.T)  # [V, H]
    bias_tiled = np.ascontiguousarray(np.tile(b, (128, J)))  # [128, J*H]
    flat = src.reshape(-1).astype(np.int16)  # V = 32000 < 2^15
    in_maps = []
    for c in range(N_CORES):
        tok = flat[c * TOK : (c + 1) * TOK]
        # gather position i = j*128 + p must fetch token 4p + j
        g = tok.reshape(128, J).T.reshape(-1)
        # dma_gather index layout: idx16[p16, s] = g[s*16 + p16], replicated x8
        idx16 = g.reshape(IDX_COLS, 16).T  # [16, 32]
        in_maps.append(
            {
                "wt": w_t,
                "idx": np.ascontiguousarray(np.tile(idx16, (8, 1))),
                "bias": bias_tiled,
            }
        )

    res = run_bass_kernel_spmd(nc, in_maps, list(range(N_CORES)), **spmd_kwargs)
    out = np.concatenate([res.results[c]["out"] for c in range(N_CORES)], axis=0)
    return out.reshape(B, S, H), res


def kernel(src, W, b):
    out, _ = _run(src, W, b)
    return out


# revision 5
# speedup vs baseline: 1.1112x; 1.1112x over previous
"""Embedding lookup (disguised as one-hot @ W.T + b) on 8 TRN2 NeuronCores.

Reference computes out[b,s,:] = W[:, src[b,s]] + b with
  src: [16, 256] int, W: [128, 32000] f32, b: [128] f32  ->  out [16, 256, 128] f32.

Strategy (data-parallel on batch, per the sharding hint):
  - Host: relayout W to a row-major table W_T = W.T  [V=32000, H=128] so one
    embedding row is 512 contiguous bytes; replicate W_T + bias to all cores.
  - Each of the 8 cores handles 512 tokens (2 batches). Token order is
    permuted on host so gather slot (p, j) holds token 4*p + j, making the
    final store per-partition contiguous (2 KB / partition).
  - Device: one SWDGE dma_gather pulls all 512 rows HBM->SBUF (512 B
    descriptors), one DVE add applies the bias, one HWDGE DMA stores the
    [128, 4*128] tile to the output.
"""

import sys

import numpy as np

if "/opt/trn_rl_repo" not in sys.path:
    sys.path.insert(0, "/opt/trn_rl_repo")

B, S, V, H = 16, 256, 32000, 128
N_CORES = 8
TOK = B * S // N_CORES  # 512 tokens per core
J = TOK // 128  # 4 tokens per partition
IDX_COLS = TOK // 16  # 32 int16 per partition row in the wrapped index tile

_NC_CACHE = {}


def _build_nc():
    import concourse.bacc as bacc
    import concourse.mybir as mybir

    nc = bacc.Bacc("TRN2", target_bir_lowering=False)

    wt = nc.dram_tensor("wt", [V, H], mybir.dt.float32, kind="ExternalInput")
    idx = nc.dram_tensor("idx", [128, IDX_COLS], mybir.dt.int16, kind="ExternalInput")
    bias = nc.dram_tensor("bias", [128, J * H], mybir.dt.float32, kind="ExternalInput")
    out = nc.dram_tensor("out", [TOK, H], mybir.dt.float32, kind="ExternalOutput")
    # token t = 4p + j lives at dst[p, j, :]; store is contiguous per partition.
    out_view = out[:].rearrange("(p j) h -> p (j h)", p=128)

    from concourse.library_config import mlp

    with (
        nc.sbuf_tensor("idx_sb", [128, IDX_COLS], mybir.dt.int16) as idx_sb,
        nc.sbuf_tensor("dst_sb", [128, J, H], mybir.dt.float32) as dst_sb,
        nc.sbuf_tensor("bias_sb", [128, J * H], mybir.dt.float32) as bias_sb,
        nc.semaphore("s_idx") as s_idx,
        nc.semaphore("s_bias") as s_bias,
        nc.semaphore("s_g") as s_g,
        nc.semaphore("s_v") as s_v,
        nc.semaphore("s_out") as s_out,
        nc.Block() as block,
    ):
        dst_flat = dst_sb[:].rearrange("p j h -> p (j h)")

        @block.sync
        def _(sync):
            sync.dma_start(idx_sb[:], idx[:]).then_inc(s_idx, 16)
            sync.dma_start(bias_sb[:], bias[:]).then_inc(s_bias, 16)
            sync.wait_ge(s_v, 1)
            sync.dma_start(out_view, dst_flat).then_inc(s_out, 16)
            sync.wait_ge(s_out, 16)

        @block.gpsimd
        def _(gpsimd):
            # Start the mlp Q7 library IRAM fetch (~9 us) immediately so it
            # overlaps the input DMAs instead of serializing after them.
            gpsimd.load_library(mlp)
            gpsimd.wait_ge(s_idx, 16)
            gpsimd.dma_gather(dst_sb[:], wt[:], idx_sb[:], TOK, TOK, H).then_inc(
                s_g, 16
            )

        @block.vector
        def _(vector):
            vector.wait_ge(s_bias, 16)
            vector.wait_ge(s_g, 16)
            vector.tensor_add(dst_flat, dst_flat, bias_sb[:]).then_inc(s_v, 1)

    nc.compile()
    return nc


def _run(src, W, b, **spmd_kwargs):
    from concourse.bass_utils import run_bass_kernel_spmd

    src = np.asarray(src)
    W = np.asarray(W, dtype=np.float32)
    b = np.asarray(b, dtype=np.float32)
    assert src.shape == (B, S) and W.shape == (H, V) and b.shape == (H,)

    if "nc" not in _NC_CACHE:
        _NC_CACHE["nc"] = _build_nc()
    nc = _NC_CACHE["nc"]

    # Host-side sharding / layout prep.
    w_t = np.ascontiguousarray(W.T)  # [V, H]
    bias_tiled = np.ascontiguousarray(np.tile(b, (128, J)))  # [128, J*H]
    flat = src.reshape(-1).astype(np.int16)  # V = 32000 < 2^15
    in_maps = []
    for c in range(N_CORES):
        tok = flat[c * TOK : (c + 1) * TOK]
        # gather position i = j*128 + p must fetch token 4p + j
        g = tok.reshape(128, J).T.reshape(-1)
        # dma_gather index layout: idx16[p16, s] = g[s*16 + p16], replicated x8
        idx16 = g.reshape(IDX_COLS, 16).T  # [16, 32]
        in_maps.append(
            {
                "wt": w_t,
                "idx": np.ascontiguousarray(np.tile(idx16, (8, 1))),
                "bias": bias_tiled,
            }
        )

    res = run_bass_kernel_spmd(nc, in_maps, list(range(N_CORES)), **spmd_kwargs)
    out = np.concatenate([res.results[c]["out"] for c in range(N_CORES)], axis=0)
    return out.reshape(B, S, H), res


def kernel(src, W, b):
    out, _ = _run(src, W, b)
    return out
